# revision 4
# baseline (speedup 1.0000x reference)
"""Trainium2 Bass kernel for the KalmanFilterEstimator problem.

Math
----
Reference scan (per step, carry (x, P, L)):
    x_pred = x @ Wfx + bfx + u @ Wfu + bfu + d @ Wfd + bfd
    y      = x_pred @ Wfy + bfy
    P_pred = Wfx @ (P @ Wfx^T) + Q
    x_new  = x_pred + (ym - y) @ L^T            # L from the carry (previous step)
    S_inv  = inv(R + Wfy^T @ (P_pred @ Wfy))
    L_new  = (P_pred @ Wfy) @ S_inv
    P_new  = I - L_new @ (Wfy^T @ P_pred)
Only the final x is returned.

P/L are batch-independent, so the gain sequence L_t is precomputed on host
(float64 Riccati recursion). The x recurrence is then linear:
    x_{t+1} = x_t @ G_t + h_t,
    G_t = Wfx @ M_t,  M_t = I - Wfy @ L_t^T,
    h_t = (u_t@Wfu + d_t@Wfd + b) @ M_t + (ym_t - bfy) @ L_t^T,  b = bfx+bfu+bfd.
With x_0 = 0 and suffix products S_t = G_{t+1} ... G_{T-1}:
    x_T = sum_t [ ym_t @ (L_t^T S_t) + u_t @ (Wfu M_t S_t) + d_t @ (Wfd M_t S_t) ] + c
i.e. one tall-skinny matmul  x_T^T = WB^T @ ZT  with contraction over (t, feature).

The closed loop is strongly stable (||S_t|| decays ~0.3x per step for this
problem), so ||S_t|| underflows to exact f32 zero a few dozen steps from the
end; steps with ||S_t||_2 < 1e-10 contribute < 1e-9 relative and are skipped.
The cutoff is computed from the actual weights at runtime (keep >= 128 steps,
falls back to the full sequence if the loop were ever slow to forget).

Device kernel (per core): acc(64, 256) += wb_chunk(128, 64)^T @ zt_chunk(128, 256)
accumulated in PSUM over the core's contraction rows; cores split the
contraction dim (time); host sums the 8 partials and adds the constant c.
"""
import numpy as np

NCORES = 8
PART = 128  # SBUF partitions / matmul contraction tile


def _precompute(Wfx, bfx, Wfu, bfu, Wfd, bfd, Wfy, bfy, T):
    f8 = np.float64
    Wfx = Wfx.astype(f8); Wfy = Wfy.astype(f8)
    Wfu = Wfu.astype(f8); Wfd = Wfd.astype(f8)
    b = bfx.astype(f8) + bfu.astype(f8) + bfd.astype(f8)
    bfy = bfy.astype(f8)
    nx = Wfx.shape[0]; ny = Wfy.shape[1]
    nu = Wfu.shape[0]; nd = Wfd.shape[0]
    I = np.eye(nx, dtype=f8)
    Q = np.eye(nx, dtype=f8)
    R = np.eye(ny, dtype=f8)

    Ls = np.empty((T, nx, ny), dtype=f8)
    P = np.eye(nx, dtype=f8)
    L = np.zeros((nx, ny), dtype=f8)
    for t in range(T):
        Ls[t] = L
        P = Wfx @ (P @ Wfx.T) + Q
        S_inv = np.linalg.inv(R + Wfy.T @ (P @ Wfy))
        L_new = (P @ Wfy) @ S_inv
        P = I - L_new @ (Wfy.T @ P)
        L = L_new

    Ay = np.empty((T, ny, nx), dtype=f8)
    Au = np.empty((T, nu, nx), dtype=f8)
    Ad = np.empty((T, nd, nx), dtype=f8)
    snorm = np.empty(T, dtype=f8)
    c = np.zeros(nx, dtype=f8)
    S = np.eye(nx, dtype=f8)
    for t in range(T - 1, -1, -1):
        M = I - Wfy @ Ls[t].T
        MS = M @ S
        LTS = Ls[t].T @ S
        Ay[t] = LTS
        Au[t] = Wfu @ MS
        Ad[t] = Wfd @ MS
        c += b @ MS - bfy @ LTS
        snorm[t] = np.linalg.norm(S, 2)
        S = (Wfx @ M) @ S
    return Ay, Au, Ad, c, snorm


def _build_bass(kc, nb, nx):
    """Per-core program: acc(nx, nb) = sum over 128-row chunks of
    zw[:, nb:nb+nx]^T @ zw[:, 0:nb], where zw (kc, nb+nx) packs the moving
    (zt) and stationary (wb) operands side by side so each chunk group
    arrives in ONE DMA.

    Raw Bass (no TileContext): this walrus build rejects instructions with
    more than ~one sync wait ("Too many sync wait commands"), which Tile's
    closing Drain (4 waits) trips. With explicit semaphores every
    instruction carries at most one wait.
    """
    import concourse.bass as bass
    import concourse.mybir as mybir

    f32 = mybir.dt.float32
    nf2 = nb + nx
    nc = bass.Bass()
    zw = nc.dram_tensor("zw", [kc, nf2], f32, kind="ExternalInput")
    acc = nc.dram_tensor("acc", [nx, nb], f32, kind="ExternalOutput")
    nchunks = kc // PART
    # chunks per SBUF group: cap the group tile at ~46 KiB per partition
    G = max(1, min(nchunks, (46 * 1024) // (nf2 * 4)))
    groups = []
    off = 0
    while off < nchunks:
        groups.append((off, min(G, nchunks - off)))
        off += groups[-1][1]
    ng = len(groups)
    zwv = zw.rearrange("(c p) n -> p c n", p=PART)  # (128, nchunks, nf2)

    with (
        nc.sbuf_tensor([PART, G, nf2], f32) as zwt0,
        nc.sbuf_tensor([PART, G, nf2], f32) as zwt1,
        nc.sbuf_tensor([nx, nb], f32) as outt,
        nc.psum_tensor([nx, nb], f32) as ps,
        nc.semaphore() as dsem0,   # group-load completions, even groups
        nc.semaphore() as dsem1,   # group-load completions, odd groups
        nc.semaphore() as psem,    # PE groups retired
        nc.semaphore() as vsem,    # PSUM->SBUF copy done
        nc.semaphore() as osem,    # output DMA done
        nc.Block() as block,
    ):
        tiles = [zwt0, zwt1]
        dsems = [dsem0, dsem1]

        @block.sync
        def _(sync):
            for g, (off, gsz) in enumerate(groups):
                if g >= 2:
                    # WAR: tile g%2 is reused; its previous group's matmuls
                    # must have retired first
                    sync.wait_ge(psem, g - 1)
                sync.dma_start(
                    tiles[g % 2][:, 0:gsz, :], zwv[:, off:off + gsz, :]
                ).then_inc(dsems[g % 2], 16)
            sync.wait_ge(vsem, 1)
            sync.dma_start(acc[:, :], outt[:]).then_inc(osem, 16)
            sync.wait_ge(osem, 16)  # keep SP alive until the result landed

        @block.tensor
        def _(tensor):
            i = 0
            for g, (off, gsz) in enumerate(groups):
                tensor.wait_ge(dsems[g % 2], 16 * (g // 2 + 1))
                t = tiles[g % 2]
                for j in range(gsz):
                    mm = nc.tensor.matmul(
                        ps[:], t[:, j, nb:nf2], t[:, j, 0:nb],
                        start=(i == 0), stop=(i == nchunks - 1))
                    i += 1
                    if j == gsz - 1:
                        mm.then_inc(psem, 1)

        @block.vector
        def _(vector):
            vector.wait_ge(psem, ng)
            nc.vector.tensor_copy(outt[:], ps[:]).then_inc(vsem, 1)

    return nc


def _prepare(inputs):
    """Host precompute + data marshalling. Returns (in_maps, nc, cvec, meta)."""
    Ym = np.asarray(inputs["Ym"]); U = np.asarray(inputs["U"]); D = np.asarray(inputs["D"])
    T, B, ny = Ym.shape
    nu = U.shape[2]; nd = D.shape[2]
    nx = np.asarray(inputs["Wfx"]).shape[0]
    nf = ny + nu + nd

    Ay, Au, Ad, cvec, snorm = _precompute(
        np.asarray(inputs["Wfx"]), np.asarray(inputs["bfx"]),
        np.asarray(inputs["Wfu"]), np.asarray(inputs["bfu"]),
        np.asarray(inputs["Wfd"]), np.asarray(inputs["bfd"]),
        np.asarray(inputs["Wfy"]), np.asarray(inputs["bfy"]), T)

    # steps with ||S_t|| < 1e-10 contribute < ~1e-9 relative; keep a 64-step
    # margin and round so each core's row count is a multiple of 128
    cut = int(np.argmax(snorm > 1e-10))
    keep = T - cut + 64
    step_quantum = (NCORES * PART) // np.gcd(NCORES * PART, nf)
    keep = min(T, -(-keep // step_quantum) * step_quantum)
    s = T - keep

    Z = np.concatenate([Ym[s:], U[s:], D[s:]], axis=2)          # (keep, B, nf)
    ZT = np.ascontiguousarray(Z.transpose(0, 2, 1)).reshape(keep * nf, B)
    ZT = ZT.astype(np.float32, copy=False)
    WB = np.concatenate([Ay[s:], Au[s:], Ad[s:]], axis=1).reshape(keep * nf, nx)
    WB = WB.astype(np.float32)
    # pack moving + stationary operands side by side: (K, B+nx)
    ZW = np.concatenate([ZT, WB], axis=1)

    kc = (keep * nf) // NCORES
    assert kc % PART == 0, (keep, nf, kc)
    in_maps = [
        {"zw": np.ascontiguousarray(ZW[c * kc:(c + 1) * kc])}
        for c in range(NCORES)
    ]
    nc = _build_bass(kc, B, nx)
    return in_maps, nc, cvec, dict(keep=keep, kc=kc, B=B, nx=nx)


def _finish(results, cvec):
    accT = np.zeros_like(results[0]["acc"], dtype=np.float64)
    for r in results:
        accT += r["acc"]
    return (accT.T + cvec).astype(np.float32)


def kernel(**inputs):
    from concourse.bass_utils import run_bass_kernel_spmd
    in_maps, nc, cvec, _ = _prepare(inputs)
    res = run_bass_kernel_spmd(nc, in_maps, core_ids=list(range(NCORES)))
    return _finish(res.results, cvec)


# revision 7
# speedup vs baseline: 1.1713x; 1.1713x over previous
"""Trainium2 Bass kernel for the KalmanFilterEstimator problem.

Math
----
Reference scan (per step, carry (x, P, L)):
    x_pred = x @ Wfx + bfx + u @ Wfu + bfu + d @ Wfd + bfd
    y      = x_pred @ Wfy + bfy
    P_pred = Wfx @ (P @ Wfx^T) + Q
    x_new  = x_pred + (ym - y) @ L^T            # L from the carry (previous step)
    S_inv  = inv(R + Wfy^T @ (P_pred @ Wfy))
    L_new  = (P_pred @ Wfy) @ S_inv
    P_new  = I - L_new @ (Wfy^T @ P_pred)
Only the final x is returned.

P/L are batch-independent, so the gain sequence L_t is precomputed on host
(float64 Riccati recursion). The x recurrence is then linear:
    x_{t+1} = x_t @ G_t + h_t,
    G_t = Wfx @ M_t,  M_t = I - Wfy @ L_t^T,
    h_t = (u_t@Wfu + d_t@Wfd + b) @ M_t + (ym_t - bfy) @ L_t^T,  b = bfx+bfu+bfd.
With x_0 = 0 and suffix products S_t = G_{t+1} ... G_{T-1}:
    x_T = sum_t [ ym_t @ (L_t^T S_t) + u_t @ (Wfu M_t S_t) + d_t @ (Wfd M_t S_t) ] + c
i.e. one tall-skinny matmul  x_T^T = WB^T @ ZT  with contraction over (t, feature).

The closed loop is strongly stable (||S_t|| decays ~0.3x per step for this
problem), so ||S_t|| underflows to exact f32 zero a few dozen steps from the
end; steps with ||S_t||_2 < 1e-10 contribute < 1e-9 relative and are skipped.
The cutoff is computed from the actual weights at runtime (keep >= 128 steps,
falls back to the full sequence if the loop were ever slow to forget).

Device kernel (per core): acc(64, 256) += wb_chunk(128, 64)^T @ zt_chunk(128, 256)
accumulated in PSUM over the core's contraction rows; cores split the
contraction dim (time); host sums the 8 partials and adds the constant c.
"""
import numpy as np

NCORES = 8
PART = 128  # SBUF partitions / matmul contraction tile


def _precompute(Wfx, bfx, Wfu, bfu, Wfd, bfd, Wfy, bfy, T):
    f8 = np.float64
    Wfx = Wfx.astype(f8); Wfy = Wfy.astype(f8)
    Wfu = Wfu.astype(f8); Wfd = Wfd.astype(f8)
    b = bfx.astype(f8) + bfu.astype(f8) + bfd.astype(f8)
    bfy = bfy.astype(f8)
    nx = Wfx.shape[0]; ny = Wfy.shape[1]
    nu = Wfu.shape[0]; nd = Wfd.shape[0]
    I = np.eye(nx, dtype=f8)
    Q = np.eye(nx, dtype=f8)
    R = np.eye(ny, dtype=f8)

    Ls = np.empty((T, nx, ny), dtype=f8)
    P = np.eye(nx, dtype=f8)
    L = np.zeros((nx, ny), dtype=f8)
    for t in range(T):
        Ls[t] = L
        P = Wfx @ (P @ Wfx.T) + Q
        S_inv = np.linalg.inv(R + Wfy.T @ (P @ Wfy))
        L_new = (P @ Wfy) @ S_inv
        P = I - L_new @ (Wfy.T @ P)
        L = L_new

    Ay = np.empty((T, ny, nx), dtype=f8)
    Au = np.empty((T, nu, nx), dtype=f8)
    Ad = np.empty((T, nd, nx), dtype=f8)
    snorm = np.empty(T, dtype=f8)
    c = np.zeros(nx, dtype=f8)
    S = np.eye(nx, dtype=f8)
    for t in range(T - 1, -1, -1):
        M = I - Wfy @ Ls[t].T
        MS = M @ S
        LTS = Ls[t].T @ S
        Ay[t] = LTS
        Au[t] = Wfu @ MS
        Ad[t] = Wfd @ MS
        c += b @ MS - bfy @ LTS
        snorm[t] = np.linalg.norm(S, 2)
        S = (Wfx @ M) @ S
    return Ay, Au, Ad, c, snorm


def _build_bass(kc, nb, nx):
    """Per-core program: acc(nx, nb) = sum over 128-row chunks of
    zw[:, nb:nb+nx]^T @ zw[:, 0:nb], where zw (kc, nb+nx) packs the moving
    (zt) and stationary (wb) operands side by side so each chunk group
    arrives in ONE DMA.

    Raw Bass (no TileContext): this walrus build rejects instructions with
    more than ~one sync wait ("Too many sync wait commands"), which Tile's
    closing Drain (4 waits) trips. With explicit semaphores every
    instruction carries at most one wait.
    """
    import concourse.bass as bass
    import concourse.mybir as mybir

    f32 = mybir.dt.float32
    nf2 = nb + nx
    nc = bass.Bass()
    zw = nc.dram_tensor("zw", [kc, nf2], f32, kind="ExternalInput")
    acc = nc.dram_tensor("acc", [nx, nb], f32, kind="ExternalOutput")
    nchunks = kc // PART
    NSLOT = min(nchunks, 8)  # in-flight chunk slots (each its own DMA queue)

    with (
        nc.sbuf_tensor([PART, NSLOT, nf2], f32) as zwt,
        nc.sbuf_tensor([nx, nb], f32) as outt,
        nc.psum_tensor([nx, nb], f32) as ps,
        nc.Block() as block,
        # per-slot DMA-completion sems so each matmul carries exactly one wait
        _multisem(nc, NSLOT) as dsems,
        nc.semaphore() as psem,    # matmuls retired (for slot reuse)
        nc.semaphore() as vsem,    # PSUM->SBUF copy done
        nc.semaphore() as osem,    # output DMA done
    ):
        @block.sync
        def _(sync):
            for i in range(nchunks):
                s = i % NSLOT
                if i >= NSLOT:
                    # WAR: slot s reused; its previous chunk's matmul retired?
                    sync.wait_ge(psem, i - NSLOT + 1)
                sync.dma_start(
                    zwt[:, s, :], zw[i * PART:(i + 1) * PART, :]
                ).then_inc(dsems[s], 16)
            sync.wait_ge(vsem, 1)
            sync.dma_start(acc[:, :], outt[:]).then_inc(osem, 16)
            sync.wait_ge(osem, 16)  # keep SP alive until the result landed

        @block.tensor
        def _(tensor):
            for i in range(nchunks):
                s = i % NSLOT
                tensor.wait_ge(dsems[s], 16 * (i // NSLOT + 1))
                nc.tensor.matmul(
                    ps[:], zwt[:, s, nb:nf2], zwt[:, s, 0:nb],
                    start=(i == 0), stop=(i == nchunks - 1),
                ).then_inc(psem, 1)

        @block.vector
        def _(vector):
            vector.wait_ge(psem, nchunks)
            nc.vector.tensor_copy(outt[:], ps[:]).then_inc(vsem, 1)

    return nc


def _multisem(nc, n):
    from contextlib import ExitStack, contextmanager

    @contextmanager
    def _cm():
        with ExitStack() as es:
            yield [es.enter_context(nc.semaphore(f"dsem{i}")) for i in range(n)]
    return _cm()


def _prepare(inputs):
    """Host precompute + data marshalling. Returns (in_maps, nc, cvec, meta)."""
    Ym = np.asarray(inputs["Ym"]); U = np.asarray(inputs["U"]); D = np.asarray(inputs["D"])
    T, B, ny = Ym.shape
    nu = U.shape[2]; nd = D.shape[2]
    nx = np.asarray(inputs["Wfx"]).shape[0]
    nf = ny + nu + nd

    Ay, Au, Ad, cvec, snorm = _precompute(
        np.asarray(inputs["Wfx"]), np.asarray(inputs["bfx"]),
        np.asarray(inputs["Wfu"]), np.asarray(inputs["bfu"]),
        np.asarray(inputs["Wfd"]), np.asarray(inputs["bfd"]),
        np.asarray(inputs["Wfy"]), np.asarray(inputs["bfy"]), T)

    # steps with ||S_t|| < 1e-10 contribute < ~1e-9 relative; keep a 64-step
    # margin and round so each core's row count is a multiple of 128
    cut = int(np.argmax(snorm > 1e-10))
    keep = T - cut + 64
    step_quantum = (NCORES * PART) // np.gcd(NCORES * PART, nf)
    keep = min(T, -(-keep // step_quantum) * step_quantum)
    s = T - keep

    Z = np.concatenate([Ym[s:], U[s:], D[s:]], axis=2)          # (keep, B, nf)
    ZT = np.ascontiguousarray(Z.transpose(0, 2, 1)).reshape(keep * nf, B)
    ZT = ZT.astype(np.float32, copy=False)
    WB = np.concatenate([Ay[s:], Au[s:], Ad[s:]], axis=1).reshape(keep * nf, nx)
    WB = WB.astype(np.float32)
    # pack moving + stationary operands side by side: (K, B+nx)
    ZW = np.concatenate([ZT, WB], axis=1)

    kc = (keep * nf) // NCORES
    assert kc % PART == 0, (keep, nf, kc)
    in_maps = [
        {"zw": np.ascontiguousarray(ZW[c * kc:(c + 1) * kc])}
        for c in range(NCORES)
    ]
    nc = _build_bass(kc, B, nx)
    return in_maps, nc, cvec, dict(keep=keep, kc=kc, B=B, nx=nx)


def _finish(results, cvec):
    accT = np.zeros_like(results[0]["acc"], dtype=np.float64)
    for r in results:
        accT += r["acc"]
    return (accT.T + cvec).astype(np.float32)


def kernel(**inputs):
    from concourse.bass_utils import run_bass_kernel_spmd
    in_maps, nc, cvec, _ = _prepare(inputs)
    res = run_bass_kernel_spmd(nc, in_maps, core_ids=list(range(NCORES)))
    return _finish(res.results, cvec)


# revision 12
# speedup vs baseline: 1.2060x; 1.0296x over previous
"""Trainium2 Bass kernel for the KalmanFilterEstimator problem.

Math
----
Reference scan (per step, carry (x, P, L)):
    x_pred = x @ Wfx + bfx + u @ Wfu + bfu + d @ Wfd + bfd
    y      = x_pred @ Wfy + bfy
    P_pred = Wfx @ (P @ Wfx^T) + Q
    x_new  = x_pred + (ym - y) @ L^T            # L from the carry (previous step)
    S_inv  = inv(R + Wfy^T @ (P_pred @ Wfy))
    L_new  = (P_pred @ Wfy) @ S_inv
    P_new  = I - L_new @ (Wfy^T @ P_pred)
Only the final x is returned.

P/L are batch-independent, so the gain sequence L_t is precomputed on host
(float64 Riccati recursion). The x recurrence is then linear:
    x_{t+1} = x_t @ G_t + h_t,
    G_t = Wfx @ M_t,  M_t = I - Wfy @ L_t^T,
    h_t = (u_t@Wfu + d_t@Wfd + b) @ M_t + (ym_t - bfy) @ L_t^T,  b = bfx+bfu+bfd.
With x_0 = 0 and suffix products S_t = G_{t+1} ... G_{T-1}:
    x_T = sum_t [ ym_t @ (L_t^T S_t) + u_t @ (Wfu M_t S_t) + d_t @ (Wfd M_t S_t) ] + c
i.e. one tall-skinny matmul  x_T^T = WB^T @ ZT  with contraction over (t, feature).

The closed loop is strongly stable (||S_t|| decays ~0.3x per step for this
problem), so ||S_t|| underflows to exact f32 zero a few dozen steps from the
end; steps with ||S_t||_2 < 1e-10 contribute < 1e-9 relative and are skipped.
The cutoff is computed from the actual weights at runtime (keep >= 128 steps,
falls back to the full sequence if the loop were ever slow to forget).

Device kernel (per core): acc(64, 256) += wb_chunk(128, 64)^T @ zt_chunk(128, 256)
accumulated in PSUM over the core's contraction rows; cores split the
contraction dim (time); host sums the 8 partials and adds the constant c.
"""
import numpy as np

NCORES = 8
PART = 128  # SBUF partitions / matmul contraction tile
USE_F32R = False  # single-pass PE fp32 mode (validated per-problem before enabling)


def _precompute(Wfx, bfx, Wfu, bfu, Wfd, bfd, Wfy, bfy, T):
    f8 = np.float64
    Wfx = Wfx.astype(f8); Wfy = Wfy.astype(f8)
    Wfu = Wfu.astype(f8); Wfd = Wfd.astype(f8)
    b = bfx.astype(f8) + bfu.astype(f8) + bfd.astype(f8)
    bfy = bfy.astype(f8)
    nx = Wfx.shape[0]; ny = Wfy.shape[1]
    nu = Wfu.shape[0]; nd = Wfd.shape[0]
    I = np.eye(nx, dtype=f8)
    Q = np.eye(nx, dtype=f8)
    R = np.eye(ny, dtype=f8)

    Ls = np.empty((T, nx, ny), dtype=f8)
    P = np.eye(nx, dtype=f8)
    L = np.zeros((nx, ny), dtype=f8)
    for t in range(T):
        Ls[t] = L
        P = Wfx @ (P @ Wfx.T) + Q
        S_inv = np.linalg.inv(R + Wfy.T @ (P @ Wfy))
        L_new = (P @ Wfy) @ S_inv
        P = I - L_new @ (Wfy.T @ P)
        L = L_new

    Ay = np.empty((T, ny, nx), dtype=f8)
    Au = np.empty((T, nu, nx), dtype=f8)
    Ad = np.empty((T, nd, nx), dtype=f8)
    snorm = np.empty(T, dtype=f8)
    c = np.zeros(nx, dtype=f8)
    S = np.eye(nx, dtype=f8)
    for t in range(T - 1, -1, -1):
        M = I - Wfy @ Ls[t].T
        MS = M @ S
        LTS = Ls[t].T @ S
        Ay[t] = LTS
        Au[t] = Wfu @ MS
        Ad[t] = Wfd @ MS
        c += b @ MS - bfy @ LTS
        snorm[t] = np.linalg.norm(S, 2)
        S = (Wfx @ M) @ S
    return Ay, Au, Ad, c, snorm


def _build_bass(kc, nb, nx, use_f32r=False):
    """Per-core program: acc(nx, nb) = sum over 128-row chunks of
    zw[:, nb:nb+nx]^T @ zw[:, 0:nb], where zw (kc, nb+nx) packs the moving
    (zt) and stationary (wb) operands side by side so each chunk group
    arrives in ONE DMA.

    Raw Bass (no TileContext): this walrus build rejects instructions with
    more than ~one sync wait ("Too many sync wait commands"), which Tile's
    closing Drain (4 waits) trips. With explicit semaphores every
    instruction carries at most one wait.
    """
    import concourse.bass as bass
    import concourse.mybir as mybir

    f32 = mybir.dt.float32
    # float32r: same fp32 bytes, single-pass PE matmul (1 cyc/row at N>=256)
    # instead of the two-pass LOW/HIGH fp32 decomposition
    mmdt = mybir.dt.float32r if use_f32r else f32
    nf2 = nb + nx
    nc = bass.Bass()
    zw = nc.dram_tensor("zw", [kc, nf2], mmdt, kind="ExternalInput")
    acc = nc.dram_tensor("acc", [nx, nb], f32, kind="ExternalOutput")
    nchunks = kc // PART
    NSLOT = min(nchunks, 8)  # in-flight chunk slots (each its own DMA queue)

    with (
        nc.sbuf_tensor([PART, NSLOT, nf2], mmdt) as zwt,
        nc.sbuf_tensor([nx, nb], f32) as outt,
        nc.psum_tensor([nx, nb], f32) as ps,
        nc.Block() as block,
        # per-slot DMA-completion sems so each matmul carries exactly one wait
        _multisem(nc, NSLOT) as dsems,
        nc.semaphore() as psem,    # matmuls retired (for slot reuse)
        nc.semaphore() as vsem,    # PSUM->SBUF copy done
        nc.semaphore() as osem,    # output DMA done
    ):
        @block.sync
        def _(sync):
            for i in range(nchunks):
                s = i % NSLOT
                if i >= NSLOT:
                    # WAR: slot s reused; its previous chunk's matmul retired?
                    sync.wait_ge(psem, i - NSLOT + 1)
                sync.dma_start(
                    zwt[:, s, :], zw[i * PART:(i + 1) * PART, :]
                ).then_inc(dsems[s], 16)
            sync.wait_ge(vsem, 1)
            sync.dma_start(acc[:, :], outt[:]).then_inc(osem, 16)
            sync.wait_ge(osem, 16)  # keep SP alive until the result landed

        @block.tensor
        def _(tensor):
            for i in range(nchunks):
                s = i % NSLOT
                tensor.wait_ge(dsems[s], 16 * (i // NSLOT + 1))
                nc.tensor.matmul(
                    ps[:], zwt[:, s, nb:nf2], zwt[:, s, 0:nb],
                    start=(i == 0), stop=(i == nchunks - 1),
                ).then_inc(psem, 1)

        @block.vector
        def _(vector):
            vector.wait_ge(psem, nchunks)
            nc.vector.tensor_copy(outt[:], ps[:]).then_inc(vsem, 1)

    return nc


def _multisem(nc, n):
    from contextlib import ExitStack, contextmanager

    @contextmanager
    def _cm():
        with ExitStack() as es:
            yield [es.enter_context(nc.semaphore(f"dsem{i}")) for i in range(n)]
    return _cm()


def _prepare(inputs):
    """Host precompute + data marshalling. Returns (in_maps, nc, cvec, meta)."""
    Ym = np.asarray(inputs["Ym"]); U = np.asarray(inputs["U"]); D = np.asarray(inputs["D"])
    T, B, ny = Ym.shape
    nu = U.shape[2]; nd = D.shape[2]
    nx = np.asarray(inputs["Wfx"]).shape[0]
    nf = ny + nu + nd

    Ay, Au, Ad, cvec, snorm = _precompute(
        np.asarray(inputs["Wfx"]), np.asarray(inputs["bfx"]),
        np.asarray(inputs["Wfu"]), np.asarray(inputs["bfu"]),
        np.asarray(inputs["Wfd"]), np.asarray(inputs["bfd"]),
        np.asarray(inputs["Wfy"]), np.asarray(inputs["bfy"]), T)

    # steps with ||S_t|| < 1e-10 contribute < ~1e-9 relative; keep a 64-step
    # margin and round so each core's row count is a multiple of 128
    cut = int(np.argmax(snorm > 1e-10))
    keep = T - cut + 64
    step_quantum = (NCORES * PART) // np.gcd(NCORES * PART, nf)
    keep = min(T, -(-keep // step_quantum) * step_quantum)
    s = T - keep

    Z = np.concatenate([Ym[s:], U[s:], D[s:]], axis=2)          # (keep, B, nf)
    ZT = np.ascontiguousarray(Z.transpose(0, 2, 1)).reshape(keep * nf, B)
    ZT = ZT.astype(np.float32, copy=False)
    WB = np.concatenate([Ay[s:], Au[s:], Ad[s:]], axis=1).reshape(keep * nf, nx)
    WB = WB.astype(np.float32)
    # pack moving + stationary operands side by side: (K, B+nx)
    ZW = np.concatenate([ZT, WB], axis=1)

    kc = (keep * nf) // NCORES
    assert kc % PART == 0, (keep, nf, kc)
    in_maps = [
        {"zw": np.ascontiguousarray(ZW[c * kc:(c + 1) * kc])}
        for c in range(NCORES)
    ]
    nc = _build_bass(kc, B, nx, use_f32r=USE_F32R)
    return in_maps, nc, cvec, dict(keep=keep, kc=kc, B=B, nx=nx, f32r=USE_F32R)


def _finish(results, cvec):
    accT = np.zeros_like(results[0]["acc"], dtype=np.float64)
    for r in results:
        accT += r["acc"]
    return (accT.T + cvec).astype(np.float32)


def kernel(**inputs):
    from concourse.bass_utils import run_bass_kernel_spmd
    in_maps, nc, cvec, _ = _prepare(inputs)
    res = run_bass_kernel_spmd(nc, in_maps, core_ids=list(range(NCORES)))
    return _finish(res.results, cvec)
